# revision 54
# baseline (speedup 1.0000x reference)
"""Trainium2 Bass kernel for the Burgers PINN problem (v2).

Computes u(x) for IC/BC points and the PDE residual u_t + u*u_x - nu*u_xx
for collocation points, where u is a tanh MLP (2 -> 128 -> ... -> 1, 7
hidden-to-hidden layers).

Strategy (v2 — engine-load rebalance + lagged-z pipeline; 690us -> 482us):
  - Pure data parallelism: every core gets 1/8 of x_f AND 1/8 of each
    IC/BC set (17408 points per core); MLP weights replicated, host
    pre-converted to fp16.
  - Forward-mode Taylor streams H, X = +-u_x, Y = +-u_t, Z = +-u_xx in
    transposed layout [features, points], packed per layer into ONE
    [128, 4, T] fp16 SBUF tile so stream matmuls read contiguous slices.
  - Per hidden layer:
      a|ax|ay|az = W^T [H|X|Y|Z]     (4 PE matmuls, fp16, separate banks)
      H    = tanh(a + b)             (ACT)
      s2   = 2*AX^2 = Square(sqrt2*ax)  (ACT, psum->f16)
      s    = H^2                     (ACT Square / Pool TT per S_SCHED)
      m    = H (.) s2                (Pool TT)
      az  += -/+ I m                 (PE accumulating identity matmul ==
                                      t' = az -/+ m computed ON PE, free
                                      in the cost model: Ldweights = 0)
      X|Y  = (s-1) (.) [ax|ay]       (ONE DVE scalar_tensor_tensor, the
                                      subtract-then-mult fusion kills the
                                      separate g = s-1 op)
      Z    = (s-1) (.) az'           (DVE scalar_tensor_tensor)
    The +-I alternation tracks the Z-stream sign parity tau_l = (-1)^l;
    the final parity is folded into a negated W_out column.
  - LAGGED Z-PIPELINE (the big scheduling win): mm_z/mm_t/z-stt of layer
    l-ZLAG are emitted during stage l, so every deferred op's inputs are
    stages old and the in-order engine queues never block on them; the
    z-stts execute at slot start while tanh/s of the current layer are
    still being produced.  pz is a 2-slot psum ring shared by both
    parities; psum = pa x2 + pxy(2 banks) x2 + pz x2 = 8 banks.
  - Input layer: X0/Y0/tq seeds via two-scalar tensor_scalar
    ((s0-1)*col in one 4x op); z0 = tq (.) H0 on Pool; x DMA prefetched
    two tiles ahead to hide the ~2us DMA+sem-propagation latency.
  - Output: 4 accumulating sparse matmuls -> [4,T] psum -> ACT copy -> DMA.
  - Steady state is DVE-load-bound: 1192+658 ns of mandatory 1x
    psum-crossing stt work per layer-tile (engine busy: DVE ~433us,
    ACT ~390us, Pool ~350us, PE ~270us).
"""

import sys

if "/opt/trn_rl_repo" not in sys.path:
    sys.path.insert(0, "/opt/trn_rl_repo")

import numpy as np

N_CORES = 8
H = 128
L = 7  # hidden-to-hidden layers
NF, N0, NB = 131072, 4096, 2048
NF_C, N0_C, NB_C = NF // N_CORES, N0 // N_CORES, NB // N_CORES
NPTS = N0_C + 2 * NB_C + NF_C  # 17408 points per core
TILE = 512
NTILES = NPTS // TILE  # 34
NU = 0.01 / np.pi

# consts tensor layout (columns of a [128, NCONST] fp32 array)
IBH = 0            # cols 0..6   : b_hid[l]
IB_IN = 7          # col  7      : b_in
IAX = 8            # col  8      : W_in[0, :]        (d a0/dx per partition)
IAT = 9            # col  9      : W_in[1, :]        (d a0/dt per partition)
IAX2 = 10          # col 10      : 2 * W_in[0,:]^2
INAX = 11          # col 11      : -W_in[0, :]
INAT = 12          # col 12      : -W_in[1, :]
INAX2 = 13         # col 13      : -2 * W_in[0,:]^2
NCONST = 14

# engine placement knobs: for hidden layer (t, l) use key = (t*7+l) % len()
S_SCHED = ("A", "P", "A", "P", "A", "P", "A", "A")   # s = H^2: ACT or Pool
M_SCHED = ("P",)                                     # m = H*s2: Pool or DVE
# z = (s-1)*t': "V" = DVE stt from psum; "P" = ACT copy t'->f16 then
# Pool TT(g, t') with g = s-1 materialized by a cheap 4x tensor-scalar.
Z_SCHED = ("V",)
# WIDE_Z: single [H,3,T] psum tile [ax|ay|t'] consumed by ONE 3-wide stt
WIDE_Z = False
# LAG_Z: emit mm_z/mm_t/z-stt of layer l-1 during stage l (one-stage lag)
# so they never wait mid-stage; pz becomes a 2-slot ring shared by parities.
LAG_Z = True
ZLAG = 2
S0_ENG = "D"
# seed engines for (X0, Y0, tq): "D" = DVE ts, "A" = ACT Identity
SEED_ENG = ("D", "D", "D")
AUX_FIRST = False
AUX_MID = False
STRIDE = (4,)

TRACE = False
LAST_RESULTS = None

_CACHE = {}


def _build_bass():
    import concourse.tile as tile
    from concourse import bacc, mybir

    f32 = mybir.dt.float32
    f16 = mybir.dt.float16
    AF = mybir.ActivationFunctionType
    OP = mybir.AluOpType
    SQRT2 = float(np.sqrt(2.0))

    nc = bacc.Bacc("TRN2", target_bir_lowering=False,
                   detect_race_conditions=False)

    xT = nc.dram_tensor("xt", [2, NPTS], f16, kind="ExternalInput")
    whid = nc.dram_tensor("whid", [L, H, H], f16, kind="ExternalInput")
    win = nc.dram_tensor("win", [2, H], f16, kind="ExternalInput")
    iden = nc.dram_tensor("iden", [H, 2 * H], f16, kind="ExternalInput")
    wfin = nc.dram_tensor("wfin", [H, 16], f16, kind="ExternalInput")
    consts = nc.dram_tensor("consts", [H, NCONST], f32, kind="ExternalInput")
    out4 = nc.dram_tensor("out4", [4, NPTS], f32, kind="ExternalOutput")

    with tile.TileContext(nc) as tc:
        with (
            tc.tile_pool(name="wpool", bufs=1) as wp,
            tc.tile_pool(name="spool", bufs=15) as sp,
            tc.tile_pool(name="tpool", bufs=16) as tp,
            tc.tile_pool(name="ppool", bufs=1, space="PSUM") as pp,
        ):
            # DMA emission order matters: HWDGE serializes descriptor
            # generation at 625ns each, so only what the FIRST pipeline
            # stages need (win, consts) goes before the first x tiles;
            # the bulk weight loads are emitted later (see below).
            w_r = wp.tile([H, L * H], f16, tag="whid")
            win_sb = wp.tile([2, H], f16, tag="win")
            nc.sync.dma_start(win_sb[:, :], win[:, :])
            i_sb = wp.tile([H, 2 * H], f16, tag="iden")
            wfin_sb = wp.tile([H, 16], f16, tag="wfin")
            c_sb = wp.tile([H, NCONST], f32, tag="consts")
            nc.sync.dma_start(c_sb[:, :], consts[:, :])

            def load_weights(skip0=False):
                for l in range((1 if skip0 else 0), L):
                    nc.sync.dma_start(w_r[:, l * H:(l + 1) * H], whid[l, :, :])
                nc.sync.dma_start(i_sb[:, :], iden[:, :])
                nc.sync.dma_start(wfin_sb[:, :], wfin[:, :])

            def col(j):
                return c_sb[:, j:j + 1]

            negI = i_sb[:, 0:H]   # -identity
            posI = i_sb[:, H:2 * H]

            # tiles 0,1 hold the 1024 IC/BC points: forward pass only.
            AUX_TILES = (N0_C + 2 * NB_C) // TILE  # = 2
            state = {}
            parity = {}

            def s_engine(i, l):
                return S_SCHED[(i * 7 + l) % len(S_SCHED)]

            def m_engine(i, l):
                return M_SCHED[(i * 7 + l) % len(M_SCHED)]

            def z_engine(i, l):
                return Z_SCHED[(i * 7 + l) % len(Z_SCHED)]

            xtiles = {}

            def fetch_x(i):
                """DMA tile i's input points into SBUF (prefetchable)."""
                if i in xtiles or i >= NTILES:
                    return
                x_t = sp.tile([2, TILE], f16, tag="xin")
                nc.sync.dma_start(x_t[:, :], xT[:, i * TILE:(i + 1) * TILE])
                xtiles[i] = x_t

            def stage_in(i, pref=None):
                """Input layer (K=2 fp16 matmul) + layer-0 stream seeds."""
                tsl = slice(i * TILE, (i + 1) * TILE)
                r = parity[i]
                aux = i < AUX_TILES
                fetch_x(i)
                x_t = xtiles.pop(i)
                if pref is not None:
                    fetch_x(pref)  # hide DMA + sem-propagation latency
                a = pp.tile([H, TILE], f32, tag=f"pa{r}")
                nc.tensor.matmul(a[:, :], win_sb[:, :], x_t[:, :],
                                 start=True, stop=True)
                yield
                S = sp.tile([H, 4, TILE], f16, tag="S")
                nc.scalar.activation(S[:, 0, :], a[:, :], AF.Tanh,
                                     bias=col(IB_IN))
                if not aux:
                    yield
                    # s0 + seed ts ops in one segment: the in-order
                    # queues run the chain back-to-back.
                    s = tp.tile([H, TILE], f16, tag="s")
                    if S0_ENG == "A":
                        nc.scalar.square(s[:, :], S[:, 0, :])
                    elif S0_ENG == "P":
                        nc.gpsimd.tensor_mul(s[:, :], S[:, 0, :], S[:, 0, :])
                    else:
                        nc.vector.tensor_mul(s[:, :], S[:, 0, :], S[:, 0, :])
                    # X0 = (s-1)*Wx, Y0 = (s-1)*Wt, tq = (s-1)*2Wx^2
                    # either DVE two-scalar ts or ACT Identity(w*s - w)
                    tq = tp.tile([H, TILE], f16, tag="tq")
                    seeds = [(S[:, 1, :], IAX, INAX), (S[:, 2, :], IAT, INAT),
                             (tq[:, :], IAX2, INAX2)]
                    for (dst, jw, jn), eng in zip(seeds, SEED_ENG):
                        if eng == "A":
                            nc.scalar.activation(dst, s[:, :], AF.Identity,
                                                 bias=col(jn), scale=col(jw))
                        else:
                            nc.vector.tensor_scalar(dst, s[:, :], 1.0,
                                                    col(jw), OP.subtract,
                                                    OP.mult)
                    yield
                    # z0 = tq (.) H0  (= +u_xx stream seed, tau_0 = +1)
                    nc.gpsimd.tensor_mul(S[:, 3, :], tq[:, :], S[:, 0, :])
                state[i] = S
                yield "stage"

            zctx = {}       # per-tile FIFO of deferred z-op contexts
            zring = [0]     # pz ring slot counter (2 slots shared by parities)

            def emit_zmm(zc):
                """PE part of deferred z: az = W Z_{j-1} -/+ m  (into pz ring)."""
                j = zc["j"]
                pz = pp.tile([H, TILE], f32, tag=f"pz{zring[0] % 2}")
                zring[0] += 1
                nc.tensor.matmul(pz[:, :], w_r[:, j * H:(j + 1) * H],
                                 zc["Sin"][:, 3, :], start=True, stop=False)
                ii = negI if (j % 2 == 0) else posI
                nc.tensor.matmul(pz[:, :], ii, zc["m"][:, :],
                                 start=False, stop=True)
                zc["pz"] = pz

            def emit_zcopy(zc):
                """Route C mid-phase: t' psum -> SBUF f16 on ACT."""
                tz = tp.tile([H, TILE], f16, tag="tz")
                nc.scalar.copy(tz[:, :], zc["pz"][:, :])
                zc["tz"] = tz

            def emit_zstt(zc):
                """Z_j final: DVE stt from psum, or 2x TT(g, tz) after copy."""
                if "tz" in zc:
                    nc.vector.tensor_mul(zc["Sout"][:, 3, :], zc["g"][:, :],
                                         zc["tz"][:, :])
                else:
                    nc.vector.scalar_tensor_tensor(
                        zc["Sout"][:, 3, :], zc["s"][:, :], 1.0,
                        zc["pz"][:, :], OP.subtract, OP.mult)

            def stage_hidden(i, l):
                r = parity[i]
                aux = i < AUX_TILES
                Sp = state[i]
                wl = w_r[:, l * H:(l + 1) * H]
                zq = zctx.setdefault(i, [])
                zc = zcm = None
                for e in list(zq):
                    age = l - e["j"]
                    if age == ZLAG:
                        zc = e          # final phase this stage
                        zq.remove(e)
                    elif age == ZLAG - 1 and e.get("C"):
                        zcm = e         # route-C mid phase: zmm + copy
                a = pp.tile([H, TILE], f32, tag=f"pa{r}")
                nc.tensor.matmul(a[:, :], wl, Sp[:, 0, :], start=True,
                                 stop=True)
                if not aux:
                    pxy = pp.tile([H, 2, TILE], f32, tag=f"pxy{r}")
                    nc.tensor.matmul(pxy[:, 0, :], wl, Sp[:, 1, :],
                                     start=True, stop=True)
                    nc.tensor.matmul(pxy[:, 1, :], wl, Sp[:, 2, :],
                                     start=True, stop=True)
                    yield
                    if zc is not None and not zc.get("C"):
                        emit_zmm(zc)  # all inputs ready: PE never waits
                yield
                S = sp.tile([H, 4, TILE], f16, tag="S")
                nc.scalar.activation(S[:, 0, :], a[:, :], AF.Tanh,
                                     bias=col(IBH + l))
                if not aux:
                    # s right behind tanh (its consumer xy-stt is the DVE
                    # critical path); s2's consumer (m) tolerates more lag.
                    s = tp.tile([H, TILE], f16, tag="s")
                    eng = s_engine(i, l)
                    if eng == "A":
                        nc.scalar.square(s[:, :], S[:, 0, :])
                    elif eng == "P":
                        nc.gpsimd.tensor_mul(s[:, :], S[:, 0, :], S[:, 0, :])
                    else:
                        nc.vector.tensor_mul(s[:, :], S[:, 0, :], S[:, 0, :])
                    yield
                    # deferred z-stt first: its inputs are a stage old, so
                    # DVE chews it while tanh/s of THIS layer still run.
                    # s2 here lands in the ACT queue after BOTH tiles'
                    # tanh+s pairs (s is on the xy critical path, s2 not).
                    if zc is not None:
                        emit_zstt(zc)
                    s2 = tp.tile([H, TILE], f16, tag="s2")
                    nc.scalar.activation(s2[:, :], pxy[:, 0, :], AF.Square,
                                         scale=SQRT2)  # = 2*AX^2
                    yield
                    # X|Y = (s-1) (.) [ax|ay]   (one fused DVE op)
                    s_b = s[:, :].unsqueeze(1).broadcast_to([H, 2, TILE])
                    nc.vector.scalar_tensor_tensor(
                        S[:, 1:3, :], s_b, 1.0, pxy[:, 0:2, :],
                        OP.subtract, OP.mult)
                    yield
                    # route-C mid phase: zmm after this stage's z-final (its
                    # Z_{j-1} input) and the copy once mm_t has landed.
                    if zcm is not None:
                        emit_zmm(zcm)
                    m = tp.tile([H, TILE], f16, tag="m")
                    if m_engine(i, l) == "P":
                        nc.gpsimd.tensor_mul(m[:, :], S[:, 0, :], s2[:, :])
                    else:
                        nc.vector.tensor_mul(m[:, :], S[:, 0, :], s2[:, :])
                    zc_new = {"j": l, "Sin": Sp, "Sout": S, "s": s, "m": m}
                    if z_engine(i, l) == "C":
                        g = tp.tile([H, TILE], f16, tag="g")
                        nc.vector.tensor_scalar_sub(g[:, :], s[:, :], 1.0)
                        zc_new["g"] = g
                        zc_new["C"] = True
                    zq.append(zc_new)
                    yield
                    if zcm is not None:
                        emit_zcopy(zcm)
                state[i] = S
                yield "stage"

            def stage_out(i):
                tsl = slice(i * TILE, (i + 1) * TILE)
                aux = i < AUX_TILES
                S = state.pop(i)
                for zc in zctx.pop(i, []):
                    if "tz" not in zc:
                        emit_zmm(zc)
                        yield
                        if zc.get("C"):
                            emit_zcopy(zc)
                            yield
                    emit_zstt(zc)
                    yield
                o = pp.tile([4, TILE], f32, tag=f"pz{zring[0] % 2}")
                zring[0] += 1
                if aux:
                    nc.tensor.matmul(o[:, :], wfin_sb[:, 0:4], S[:, 0, :],
                                     start=True, stop=True)
                else:
                    for mi in range(4):
                        nc.tensor.matmul(o[:, :], wfin_sb[:, 4 * mi:4 * (mi + 1)],
                                         S[:, mi, :], start=(mi == 0),
                                         stop=(mi == 3))
                yield
                o_t = sp.tile([4, TILE], f32, tag="ot")
                nc.scalar.copy(o_t[:, :], o[:, :])
                nc.sync.dma_start(out4[:, tsl], o_t[:, :])
                yield "stage"

            def tile_gen(i, pref=None):
                yield from stage_in(i, pref)
                for l in range(L):
                    yield from stage_hidden(i, l)
                yield from stage_out(i)

            # software-pipelined wavefront (same machinery as v1)
            if AUX_FIRST:
                order = list(range(NTILES))
            elif AUX_MID:
                # slot the cheap aux tiles between the last full tiles so the
                # drain keeps DVE work till the end
                order = (list(range(AUX_TILES, NTILES - 1)) + [0]
                         + [NTILES - 1] + [1])
            else:
                order = list(range(AUX_TILES, NTILES)) + list(range(AUX_TILES))
            starts = {}
            t0 = 0
            for k, i in enumerate(order):
                parity[i] = k % 2
                starts[i] = t0
                t0 += STRIDE[k % len(STRIDE)]

            # first x tiles ahead of the bulk weight DMAs on the HWDGE
            # queue: the input matmul + tanh only need win/consts/x, so the
            # pipeline front starts ~6us earlier.
            for k in range(min(2, NTILES)):
                fetch_x(order[k])
            nc.sync.dma_start(w_r[:, 0:H], whid[0, :, :])
            for k in range(2, min(4, NTILES)):
                fetch_x(order[k])
            load_weights(skip0=True)

            gens = []
            next_k = 0
            slot = 0
            while gens or next_k < NTILES:
                while next_k < NTILES and starts[order[next_k]] <= slot:
                    pref = order[next_k + 2] if next_k + 2 < NTILES else None
                    gens.append(tile_gen(order[next_k], pref))
                    next_k += 1
                pending = list(gens)
                while pending:
                    for gn in list(pending):
                        tok = next(gn, "done")
                        if tok == "stage":
                            pending.remove(gn)
                        elif tok == "done":
                            pending.remove(gn)
                            gens.remove(gn)
                slot += 1

    nc.compile()
    return nc


def _get_nc():
    if "nc" not in _CACHE:
        _CACHE["nc"] = _build_bass()
    return _CACHE["nc"]


def kernel(x_f, x0_cat, xb_left_cat, xb_right_cat,
           W_in, b_in, W_hid, b_hid, W_out, b_out):
    global LAST_RESULTS
    from concourse.bass_utils import run_bass_kernel_spmd

    f32, f16 = np.float32, np.float16
    x_f = np.asarray(x_f, f32)
    x0_cat = np.asarray(x0_cat, f32)
    xb_left_cat = np.asarray(xb_left_cat, f32)
    xb_right_cat = np.asarray(xb_right_cat, f32)
    W_in = np.ascontiguousarray(np.asarray(W_in, f32))
    b_in = np.asarray(b_in, f32)
    W_hid = np.ascontiguousarray(np.asarray(W_hid, f32))
    b_hid = np.asarray(b_hid, f32)
    W_out = np.asarray(W_out, f32)
    b_out = np.asarray(b_out, f32)

    consts = np.zeros((H, NCONST), f32)
    consts[:, 0:L] = b_hid.T
    consts[:, IB_IN] = b_in
    consts[:, IAX] = W_in[0]
    consts[:, IAT] = W_in[1]
    consts[:, IAX2] = 2.0 * W_in[0] ** 2
    consts[:, INAX] = -W_in[0]
    consts[:, INAT] = -W_in[1]
    consts[:, INAX2] = -2.0 * W_in[0] ** 2
    consts = np.ascontiguousarray(consts)

    # final sparse matmuls: mat m has +-W_out in col m (Z parity tau_7 = -1)
    wfin = np.zeros((H, 16), f16)
    for mi in range(4):
        sgn = -1.0 if mi == 3 else 1.0
        wfin[:, 4 * mi + mi] = (sgn * W_out[:, 0]).astype(f16)

    iden = np.zeros((H, 2 * H), f16)
    iden[:, 0:H] = -np.eye(H, dtype=f16)
    iden[:, H:2 * H] = np.eye(H, dtype=f16)

    in_maps = []
    for k in range(N_CORES):
        pts = np.concatenate([
            x0_cat[k * N0_C:(k + 1) * N0_C],
            xb_left_cat[k * NB_C:(k + 1) * NB_C],
            xb_right_cat[k * NB_C:(k + 1) * NB_C],
            x_f[k * NF_C:(k + 1) * NF_C],
        ], axis=0)  # [NPTS, 2]
        in_maps.append({
            "xt": np.ascontiguousarray(pts.T.astype(f16)),
            "whid": np.ascontiguousarray(W_hid.astype(f16)),
            "win": np.ascontiguousarray(W_in.astype(f16)),
            "iden": np.ascontiguousarray(iden),
            "wfin": np.ascontiguousarray(wfin),
            "consts": consts,
        })

    nc = _get_nc()
    res = run_bass_kernel_spmd(nc, in_maps, core_ids=list(range(N_CORES)),
                               trace=TRACE)
    LAST_RESULTS = res

    u0_parts, ubl_parts, ubr_parts, r_parts = [], [], [], []
    for k in range(N_CORES):
        o = res.results[k]["out4"]  # [4, NPTS]
        u = o[0] + b_out[0]
        ux, ut, uxx = o[1], o[2], o[3]
        u0_parts.append(u[:N0_C])
        ubl_parts.append(u[N0_C:N0_C + NB_C])
        ubr_parts.append(u[N0_C + NB_C:N0_C + 2 * NB_C])
        f = slice(N0_C + 2 * NB_C, None)
        r_parts.append(ut[f] + u[f] * ux[f] - NU * uxx[f])

    out = np.concatenate(u0_parts + ubl_parts + ubr_parts + r_parts)
    return np.ascontiguousarray(out.reshape(-1, 1).astype(f32))
